# revision 34
# baseline (speedup 1.0000x reference)
"""Multi-head attention (B=2, N=2048, D=1024, H=16, HD=64) on 8 TRN2 NeuronCores.

Sharding: core c handles batch b = c//4 and heads 4*(c%4) .. 4*(c%4)+3.
Each core computes the QKV projection for its 4 heads, attention, and a
partial output projection (contraction over its 256 hd-columns of w_out).
The host sums the per-core partials (y0 + y1 per core, 4 cores per batch).

V2 design (from trace analysis of the 283us baseline):
  - The kernel is two-engine-bound: Act (exp, ~142us floor) and PE
    (~136us streaming floor).  Everything else must hide under those.
  - Pair-OUTER loop (for pair: for qc:) so the pair-1 q/k projections
    have 64 kb-slots of PE slack to splice into, not 16.
  - All projection/output-projection PE work is diced into micro-tasks
    (<= ~0.45us each, the per-slot PE slack at the Act cadence) and
    popped one per kb-slot from a FIFO with eligibility times.
  - Dtypes: xT/wqk/wv/wo shipped bf16 (halves the input-DMA gate before
    the first exp); v/pt/oT bf16 (FWL weight loads for AV/outproj);
    qkT kept f32r (scores precision); y0/y1 partials bf16 (host sums f32).
  - Output projection split per head-pair into y0/y1 (single-matmul
    pieces, schedulable anywhere); host sums.
  - Every dma_start is a ~0.6us serial instruction on the SP engine, so
    trigger COUNT and ORDER are managed: segment-0's input quarters
    (qx-major xT layout) issue first, y pieces are one DMA each, and the
    softmax scale chain is 4 hops (den row -> DRAM -> [128,4] recip on
    DVE -> DRAM bf16 -> broadcast [64,512]), with the scale-mul on
    gpsimd.  Act does exp ONLY; DVE does all PSUM->SBUF copies.

Device-side layout per core:
  qkT/kT = w_qk.T @ x.T            [512, N]  (q/k per-head rows, f32r)
  v      = x @ w_v                 [N, 4*(HD+1)] bf16 (+ ones column per head)
  scoresT= kT.T-slices @ qT        [keys, queries] per head pair (row-split
           concurrent matmuls on PE row groups 0-63 / 64-127)
  pT     = exp(0.125 * scoresT)    bf16 (no max-sub: scores are O(few))
  oT|den = [v | 1].T @ pT          [65, queries] per head (row 64 = den)
  oT     = oT * (1/den)            bf16 (recip on DVE, mul on gpsimd)
  y{0,1} = oT.T-slices @ w_out     [N, D] partials per head pair
"""

import os
import sys
import types
import ctypes
import contextlib

import numpy as np
import bass_rust
import concourse.bass as bass
import concourse.tile as tile
from concourse import mybir
from concourse import bass_utils
from concourse import library_config
from concourse import masks
from concourse.vector_clock import ScopedClock


def _ensure_ntff_hook():
    """Provide antenv.axon_hooks if the container lacks it, so that
    run_bass_kernel_spmd(trace=True) works instead of raising."""
    if "antenv.axon_hooks" in sys.modules:
        return
    try:
        import antenv.axon_hooks  # noqa: F401

        return
    except ImportError:
        pass

    def _make_hook():
        so_path = "/opt/axon/libaxon_pjrt.so"
        try:
            lib = ctypes.CDLL(so_path)
        except OSError:
            return None
        if not hasattr(lib, "axon_start_nrt_profile"):
            return None
        lib.axon_start_nrt_profile.argtypes = [
            ctypes.POINTER(ctypes.c_int64),
            ctypes.c_size_t,
        ]
        lib.axon_start_nrt_profile.restype = ctypes.c_int64
        lib.axon_stop_nrt_profile.argtypes = [ctypes.c_char_p]
        lib.axon_stop_nrt_profile.restype = ctypes.c_int64

        @contextlib.contextmanager
        def _hook(output_dir, device_ids):
            import jax

            jax.devices()
            if device_ids:
                ids = (ctypes.c_int64 * len(device_ids))(*device_ids)
                rc = lib.axon_start_nrt_profile(ids, len(device_ids))
            else:
                rc = lib.axon_start_nrt_profile(None, 0)
            if rc != 0:
                raise RuntimeError(f"axon_start_nrt_profile rc={rc}")
            try:
                yield
            finally:
                lib.axon_stop_nrt_profile(str(output_dir).encode())

        return _hook

    hook = _make_hook()
    mod = types.ModuleType("antenv.axon_hooks")
    mod.get_axon_ntff_profile_hook = lambda: hook
    mod.set_axon_ntff_profile_hook = lambda h: None
    sys.modules["antenv.axon_hooks"] = mod


_ensure_ntff_hook()

B, N, D = 2, 2048, 1024
H, HD = 16, 64
HPG = 4  # heads per core
NCORES = 8
ND = D // 128  # 8 contraction chunks for the projections
NT = N // 128  # 16 token/key blocks
NQ = N // 512  # 4 query chunks

f32 = mybir.dt.float32
f32r = mybir.dt.float32r
bf16 = mybir.dt.bfloat16
EXP = mybir.ActivationFunctionType.Exp


class _TC(tile.TileContext):
    """TileContext adapted to this walrus build, which encodes at most ONE
    semaphore wait per instruction: excess waits are offloaded onto
    preceding same-engine nops, and the final drain is split the same way."""

    _ws_counter = 0

    def _lower_ordered_insts(self, ordered):
        for bbname, insts in ordered.items():
            new = []
            for inst in insts:
                si = inst.sync_info
                if (
                    si is not None
                    and len(si.on_wait) > 1
                    and inst.engine != mybir.EngineType.Unassigned
                ):
                    waits = list(si.on_wait)
                    ups = list(si.on_update)
                    for w in waits[:-1]:
                        _TC._ws_counter += 1
                        new.append(
                            mybir.InstNoOp(
                                name=f"waitsplit_{_TC._ws_counter}",
                                engine=inst.engine,
                                ins=[],
                                outs=[],
                                sync_info=bass_rust.SyncInfo(
                                    on_wait=[w], on_update=[]
                                ),
                                bass_nofuse=True,
                            )
                        )
                    inst.sync_info = bass_rust.SyncInfo(
                        on_wait=[waits[-1]], on_update=ups
                    )
                new.append(inst)
            ordered[bbname] = new
        super()._lower_ordered_insts(ordered)

    def _drain_and_barrier(self, tick_clock, wait_clock):
        nop0 = self.nc.sync.nop(nofuse=True)
        wait_clock.add_sem_waits(nop0.ins, ScopedClock({None: tick_clock.global_clock}))
        si = nop0.ins.sync_info
        waits = list(si.on_wait) if si is not None else []
        if len(waits) > 1:
            nop0.ins.sync_info = bass_rust.SyncInfo(on_wait=waits[:1], on_update=[])
            for i in range(1, len(waits)):
                n = self.nc.sync.nop(nofuse=True)
                n.ins.sync_info = bass_rust.SyncInfo(
                    on_wait=waits[i : i + 1], on_update=[]
                )
        self.nc.sync.drain()
        self.nc.all_engine_barrier()
        assert self.sems is not None
        popped = self.nc._tile_sem_poison_stack.pop()
        assert popped is self._sem_poison
        self.nc.clear_and_free_semaphores(list(self.sems.allocated().values()))
        self.nc.all_engine_barrier()


def _body(nc, tc, xT, wqk, wv, wo, y0, y1):
    with contextlib.ExitStack() as ctx:
        persist = ctx.enter_context(tc.tile_pool(name="persist", bufs=1))
        pt_pool = ctx.enter_context(tc.tile_pool(name="ptp", bufs=6))
        ysb_pool = ctx.enter_context(tc.tile_pool(name="ysbp", bufs=6))
        small = ctx.enter_context(tc.tile_pool(name="small", bufs=3))
        dscr = ctx.enter_context(tc.tile_pool(name="dscr", bufs=4, space="DRAM"))
        ps_s = ctx.enter_context(tc.tile_pool(name="ps_s", bufs=2, space="PSUM"))
        ps_o = ctx.enter_context(tc.tile_pool(name="ps_o", bufs=2, space="PSUM"))
        ps_mm = ctx.enter_context(tc.tile_pool(name="ps_mm", bufs=2, space="PSUM"))

        # ---- ACT table preload + PE clock warm-up ----
        # The exp table-set load (~1.3-2.7us) is inserted by walrus right
        # before the FIRST activation; a dummy exp on a scratch tile at t~0
        # hides the load under the input-DMA wait instead of paying it right
        # before exp(0).
        scrap = persist.tile([128, 8], f32, tag="scrap", name="scrap")
        scrap2 = persist.tile([128, 8], f32, tag="scrap2", name="scrap2")
        nc.vector.memset(scrap, 0.0)
        nc.scalar.activation(scrap2, scrap, EXP)
        # The PE HAM clock gate defaults to 1.2 GHz and only releases to
        # 2.4 GHz after ~3.4us of sustained matmul activity.  The PE sits
        # idle for ~10us waiting on input DMA anyway, so run dummy matmuls
        # on scratch tiles to warm the clock before the first real matmul
        # (the V2 trace shows the whole prefix + early slots at half clock).
        scrapW = persist.tile([128, 128], bf16, tag="scrapW", name="scrapW")
        scrapX = persist.tile([128, 512], bf16, tag="scrapX", name="scrapX")
        nc.vector.memset(scrapW, 0.0)
        nc.vector.memset(scrapX, 0.0)
        # 30 dummies: ~6 run at the cold 1.2 GHz clock (~3.5us, warming the
        # HAM), the rest at 2.4 GHz keep it warm until the input DMA lands
        # (~19us) -- a >3.4us PE idle window would re-throttle the clock
        # and the whole prefix would run at half speed (measured on V2).
        for wi in range(30):
            ps_w = ps_mm.tile([128, 512], f32, tag="mm", name=f"ps_warm{wi}")
            nc.tensor.matmul(ps_w, lhsT=scrapW, rhs=scrapX, start=True, stop=True)

        # Constants for the PE-based softmax scale chain: a [128,1] ones
        # column (sliced at the needed base partition as a 1x1 "identity"
        # for the den-row transposes, and rows 0:1 x 64 as the broadcast
        # lhsT) and a [128,128] bf16 identity for the transpose back.
        onescol = persist.tile([128, 64], bf16, tag="onescol", name="onescol")
        nc.vector.memset(onescol, 1.0)
        onescol_f = persist.tile([128, 1], f32, tag="onescol_f", name="onescol_f")
        nc.vector.memset(onescol_f, 1.0)
        ident128 = persist.tile([128, 128], bf16, tag="ident128", name="ident128")
        masks.make_identity(nc, ident128)

        # ---- persistent SBUF residents + input DMA ----
        # Every dma_start is a ~0.6us SERIAL instruction on its issuing
        # engine's queue -- the issue ORDER and engine split matter as much
        # as the transfer itself.  The critical gate for exp(0) is
        # wqk (8 chunks) + xT quarter 0 (8 chunks): spread those 16 triggers
        # across FOUR engine queues (sync/vector/tensor/gpsimd) so they are
        # all in flight by ~2.5us.  Non-critical inputs follow on sync
        # (xT q1/q2), gpsimd (xT q3) and Act (wv, wo -- idle until exp(0)).
        xT_sb, wqk_sb, wv_sb = [], [], []
        for i in range(ND):
            wqk_sb.append(persist.tile([128, 2 * HPG * HD], bf16, tag=f"wqk{i}", name=f"wqk_sb{i}"))
            wv_sb.append(persist.tile([128, HPG * HD], bf16, tag=f"wv{i}", name=f"wv_sb{i}"))
            xT_sb.append(persist.tile([128, N], bf16, tag=f"xT{i}", name=f"xT_sb{i}"))
        # Only sync/scalar/gpsimd can trigger DMAs; split the 16 critical
        # transfers across all three queues.
        for i in range(6):
            nc.sync.dma_start(out=wqk_sb[i], in_=wqk[i * 128 : (i + 1) * 128, :])
        for i in range(6, ND):
            nc.scalar.dma_start(out=wqk_sb[i], in_=wqk[i * 128 : (i + 1) * 128, :])
        for i in range(6):
            nc.gpsimd.dma_start(
                out=xT_sb[i][:, 0:512], in_=xT[i * 128 : (i + 1) * 128, 0:512]
            )
        for i in range(6, ND):
            nc.scalar.dma_start(
                out=xT_sb[i][:, 0:512], in_=xT[i * 128 : (i + 1) * 128, 0:512]
            )
        # Act: wv (needed by v(0) at ~slot 0) then wo (needed ~slot 45).
        for i in range(ND):
            nc.scalar.dma_start(out=wv_sb[i], in_=wv[i * 128 : (i + 1) * 128, :])
        wo_sb = []
        for c2 in range(2):
            t_ = persist.tile([128, D], bf16, tag=f"wo{c2}", name=f"wo_sb{c2}")
            nc.scalar.dma_start(out=t_, in_=wo[c2 * 128 : (c2 + 1) * 128, :])
            wo_sb.append(t_)
        # Sync: xT quarters 1-2 (needed from ~slot 0 resp. ~slot 3).
        for qx in range(1, 3):
            for i in range(ND):
                nc.sync.dma_start(
                    out=xT_sb[i][:, qx * 512 : (qx + 1) * 512],
                    in_=xT[i * 128 : (i + 1) * 128, qx * 512 : (qx + 1) * 512],
                )

        # qkT rows: tile 0 = qT heads 0,1 | tile 1 = qT heads 2,3
        #           tile 2 = kT heads 0,1 | tile 3 = kT heads 2,3
        # bf16: the scores matmuls stream 2x faster than f32r (saves ~29us
        # of PE), at ~0.3% extra noise on p (well inside the 2e-2 gate).
        qkT_sb = [
            persist.tile([128, N], bf16, tag=f"qkT{r}", name=f"qkT_sb{r}")
            for r in range(4)
        ]
        # v blocks with a ones column after each head: [v_h | 1] x 4
        v_sb = [
            persist.tile([128, HPG * (HD + 1)], bf16, tag=f"v{t}", name=f"v_sb{t}")
            for t in range(NT)
        ]
        oT_sb = [
            persist.tile([128, N], bf16, tag=f"oT{c2}", name=f"oT_sb{c2}")
            for c2 in range(2)
        ]
        # gpsimd queue: ones-memsets for the earliest v blocks first (v(0)
        # is consumed at slot 0), then the non-critical xT quarter 3
        # triggers (needed from ~slot 7), then the remaining memsets and
        # the ucode library for partition_broadcast + tensor_tensor divide
        # (needed from ~slot 16, the first scale chain).
        for t in range(4):
            nc.gpsimd.memset(v_sb[t], 1.0)
        for i in range(ND):
            nc.gpsimd.dma_start(
                out=xT_sb[i][:, 1536:2048],
                in_=xT[i * 128 : (i + 1) * 128, 1536:2048],
            )
        for t in range(4, NT):
            nc.gpsimd.memset(v_sb[t], 1.0)

        # ---- projection group emitters ----
        # Each emitter yields micro-steps of <= ~0.45us of PE work; the
        # final step does the PSUM->SBUF copy (on DVE).
        pools3 = (ps_mm, ps_o, ps_s)
        tags3 = ("mm", "o", "s")
        group_idx = 0

        def qk_group_steps(r, qc, force_mm=False):
            """4 matmul steps (2 accum chunks each) + 1 copy step."""
            nonlocal group_idx
            sel = 0 if force_mm else group_idx % 3
            group_idx += 1
            pool, tag = pools3[sel], tags3[sel]
            box = {}

            def mmstep(i0):
                if i0 == 0:
                    box["ps"] = pool.tile([128, 512], f32, tag=tag, name=f"ps_qk_{r}_{qc}")
                for i in range(i0, i0 + 2):
                    nc.tensor.matmul(
                        box["ps"],
                        lhsT=wqk_sb[i][:, r * 128 : (r + 1) * 128],
                        rhs=xT_sb[i][:, qc * 512 : (qc + 1) * 512],
                        start=(i == 0),
                        stop=(i == ND - 1),
                    )

            def cpstep():
                nc.vector.tensor_copy(
                    qkT_sb[r][:, qc * 512 : (qc + 1) * 512], box["ps"]
                )

            return [lambda i0=i0: mmstep(i0) for i0 in range(0, ND, 2)] + [cpstep]

        def v_group_steps(t, force_mm=False):
            """2 matmul steps (4 accum chunks each) + 1 copy step."""
            nonlocal group_idx
            sel = 0 if force_mm else group_idx % 3
            group_idx += 1
            pool, tag = pools3[sel], tags3[sel]
            box = {}

            def mmstep(i0):
                if i0 == 0:
                    box["ps"] = pool.tile([128, HPG * HD], f32, tag=tag, name=f"ps_v_{t}")
                for i in range(i0, i0 + 4):
                    nc.tensor.matmul(
                        box["ps"],
                        lhsT=xT_sb[i][:, t * 128 : (t + 1) * 128],
                        rhs=wv_sb[i],
                        start=(i == 0),
                        stop=(i == ND - 1),
                    )

            def cpstep():
                vview = v_sb[t].rearrange("p (h c) -> p h c", c=HD + 1)[:, :, 0:HD]
                nc.vector.tensor_copy(vview, box["ps"].rearrange("p (h c) -> p h c", c=HD))

            return [lambda i0=i0: mmstep(i0) for i0 in range(0, ND, 4)] + [cpstep]

        def outproj_piece(c2, t, dc, box):
            """One micro-task: single matmul + copy; the dc=1 task DMAs the
            full [128, 1024] row-block in ONE transfer (the two dc halves
            share an SBUF staging tile), halving y trigger count to 32.
            Triggers alternate sync/gpsimd rings (the scale chain no longer
            uses DMA, so both rings are safe).  Drain-wave pieces CAST via
            the Act engine (idle once the exps are done) so the tail is not
            serialized on DVE."""
            ydst = y0 if c2 == 0 else y1
            trig = nc.sync if t % 2 == 0 else nc.gpsimd

            def step():
                ps = ps_mm.tile([128, 512], f32, tag="mm", name=f"ps_y{c2}_{t}_{dc}")
                nc.tensor.matmul(
                    ps,
                    lhsT=oT_sb[c2][:, t * 128 : (t + 1) * 128],
                    rhs=wo_sb[c2][:, dc * 512 : (dc + 1) * 512],
                    start=True,
                    stop=True,
                )
                if dc == 0:
                    box["ysb"] = ysb_pool.tile(
                        [128, 1024], bf16, tag="y", name=f"ysb{c2}_{t}"
                    )
                ysb = box["ysb"]
                if box.get("act_cast"):
                    nc.scalar.copy(ysb[:, dc * 512 : (dc + 1) * 512], ps)
                else:
                    nc.vector.tensor_copy(ysb[:, dc * 512 : (dc + 1) * 512], ps)
                if dc == 1:
                    trig.dma_start(
                        out=ydst[t * 128 : (t + 1) * 128, :],
                        in_=ysb,
                    )

            return [step]

        # ---- prefix: only what slot 0's scores need ----
        # qT qc0 + kT kb0-3, gated on xT quarter 0 + wqk.  Everything else
        # (including v(0..3)) is spliced into the slot stream.
        def run_all(steps):
            for s in steps:
                s()

        run_all(qk_group_steps(0, 0))   # qT pair0 qc0
        run_all(qk_group_steps(2, 0))   # kT pair0 kb 0-3

        # ---- micro-task queue for the kb-slot loop ----
        # Tasks become eligible at a given global slot; each slot pops ALL
        # eligible tasks in eligibility order (min-heap -- a far-future
        # task must not block an overdue one behind it).
        import heapq

        taskq = []  # heap of (eligible_slot, seq, fn)
        seq_ctr = [0]

        def enq(slot, steps):
            for k, s in enumerate(steps):
                heapq.heappush(taskq, (slot + k, seq_ctr[0], s))
                seq_ctr[0] += 1

        # v blocks during early pair-0 slots.  av_pair(t) is emitted in slot
        # t+1 and pops happen before it, so v(t)'s 3 steps must be popped
        # by slot t (eligibility t-2 with pop-all-eligible).  kT qc2/qc3
        # are READ by the scores emitted at slots 8/12, which precede that
        # slot's pop -- their 5 tasks must pop by slot 4qc-1.  The early
        # slots are PE-production-bound regardless; multi-pop is correct.
        for t in range(NT):
            enq(t - 2, v_group_steps(t, force_mm=True))
        enq(-1, qk_group_steps(2, 1, force_mm=True))      # kT pair0 kb 4-7 (by slot 3)
        enq(3, qk_group_steps(2, 2, force_mm=True))       # kT pair0 kb 8-11
        enq(7, qk_group_steps(2, 3, force_mm=True))       # kT pair0 kb 12-15
        enq(11, qk_group_steps(0, 1, force_mm=True))      # qT pair0 qc1 (slot 16)
        # Pair-1 projections and qT pair0 qc2/3: confined to slots 16-44 so
        # they never share the ps_mm ring with the output-projection pieces
        # (waves start at slot 45) -- a spliced group holds its PSUM buf for
        # ~5 slots and stalls the 2-buf ring if pieces interleave.
        enq(16, qk_group_steps(0, 2, force_mm=True))      # qT pair0 qc2 (slot 32)
        enq(19, qk_group_steps(2 + 1, 0, force_mm=True))  # kT pair1
        enq(22, qk_group_steps(2 + 1, 1, force_mm=True))
        enq(25, qk_group_steps(2 + 1, 2, force_mm=True))
        enq(28, qk_group_steps(2 + 1, 3, force_mm=True))
        enq(31, qk_group_steps(0, 3, force_mm=True))      # qT pair0 qc3 (slot 48)
        enq(34, qk_group_steps(1, 0, force_mm=True))      # qT pair1
        enq(37, qk_group_steps(1, 1, force_mm=True))
        enq(40, qk_group_steps(1, 2, force_mm=True))
        enq(43, qk_group_steps(1, 3, force_mm=True))

        def pop_task(g):
            popped = False
            while taskq and taskq[0][0] <= g:
                _, _, fn = heapq.heappop(taskq)
                fn()
                popped = True
            return popped

        # ---- phase 2: attention, pair-outer ----
        def av_pair(pair, poA, poB, kb, pt):
            hA, hB = 2 * pair, 2 * pair + 1
            nc.tensor.matmul(
                poA,
                lhsT=v_sb[kb][:, hA * (HD + 1) : (hA + 1) * (HD + 1)],
                rhs=pt[:, 0:512],
                start=(kb == 0),
                stop=(kb == NT - 1),
            )
            nc.tensor.matmul(
                poB,
                lhsT=v_sb[kb][:, hB * (HD + 1) : (hB + 1) * (HD + 1)],
                rhs=pt[:, 512:1024],
                start=(kb == 0),
                stop=(kb == NT - 1),
            )

        for pair in range(2):
            for qc in range(NQ):
                seg = pair * NQ + qc
                poA = ps_o.tile([65, 512], f32, tag="o", name=f"poA_{pair}_{qc}")
                poB = ps_o.tile([65, 512], f32, tag="o", name=f"poB_{pair}_{qc}")
                pending = None
                for kb in range(NT):
                    g = seg * NT + kb
                    ps = ps_s.tile(
                        [128, 1024], f32, tag="s", name=f"ps_s_{pair}_{qc}_{kb}"
                    )
                    # Row-split concurrent pair: head A on PE rows 0-63,
                    # head B on rows 64-127.
                    nc.tensor.matmul(
                        ps[:, 0:512],
                        lhsT=qkT_sb[2 + pair][0:64, kb * 128 : (kb + 1) * 128],
                        rhs=qkT_sb[pair][0:64, qc * 512 : (qc + 1) * 512],
                        start=True,
                        stop=True,
                    )
                    nc.tensor.matmul(
                        ps[:, 512:1024],
                        lhsT=qkT_sb[2 + pair][64:128, kb * 128 : (kb + 1) * 128],
                        rhs=qkT_sb[pair][64:128, qc * 512 : (qc + 1) * 512],
                        start=True,
                        stop=True,
                    )
                    pt = pt_pool.tile(
                        [128, 1024], bf16, tag="pt", name=f"pt_{pair}_{qc}_{kb}"
                    )
                    nc.scalar.activation(pt, ps, EXP, scale=HD**-0.5)
                    pop_task(g)
                    if pending is not None:
                        av_pair(pair, poA, poB, *pending)
                    pending = (kb, pt)
                av_pair(pair, poA, poB, *pending)

                # Softmax scale chain, fully ON-CHIP (no DMA, which queued
                # ~5us/hop behind in-flight y pieces; no custom ISA, which
                # this walrus build cannot codegen):
                #   1. den row [1,512] -> [128,4] via 4 tiny PE transposes
                #      (the PE is right here at the segment end anyway)
                #   2. [128,4] reciprocal on DVE (~0.2us, vs 3.3us for the
                #      single-lane row reciprocal)
                #   3. transpose back to a [1,512] row via 4 PE transposes
                #   4. 64-partition broadcast = rank-1 matmul ones64.T @ row
                #   5. scale-mul on gpsimd
                for h, po in ((0, poA), (1, poB)):
                    qb = h * 64
                    oacc = small.tile(
                        [65, 512], f32, tag="oacc", name=f"oacc_{seg}_{h}", bufs=4
                    )
                    nc.vector.tensor_copy(oacc, po)
                    pden = ps_mm.tile([128, 4], f32, tag="mm", name=f"pden_{seg}_{h}")
                    for j in range(4):
                        nc.tensor.transpose(
                            pden[:, j : j + 1],
                            oacc[64:65, 128 * j : 128 * (j + 1)],
                            onescol_f[64:65, 0:1],
                            tile_position=(64, 0),
                        )
                    rout = small.tile([128, 4], bf16, tag="rout", name=f"rout_{seg}_{h}")
                    with nc.allow_low_precision("softmax denom recip, bf16 rep"):
                        nc.vector.reciprocal(rout, pden)
                    rrow_ps = ps_mm.tile([1, 512], bf16, tag="mm", name=f"rrowp_{seg}_{h}")
                    for j in range(4):
                        nc.tensor.transpose(
                            rrow_ps[0:1, 128 * j : 128 * (j + 1)],
                            rout[:, j : j + 1],
                            ident128,
                        )
                    rrow = small.tile([1, 512], bf16, tag="rrow", name=f"rrow_{seg}_{h}")
                    nc.vector.tensor_copy(rrow, rrow_ps)
                    rep_ps = ps_mm.tile([64, 512], f32, tag="mm", name=f"repp_{seg}_{h}")
                    nc.tensor.matmul(
                        rep_ps, lhsT=onescol[0:1, :], rhs=rrow, start=True, stop=True
                    )
                    rep = small.tile([64, 512], bf16, tag="rep", name=f"rep_{seg}_{h}")
                    nc.vector.tensor_copy(rep, rep_ps)
                    # h0's mul on gpsimd, h1's on DVE: the two run
                    # concurrently instead of serializing on gpsimd.
                    mul_eng = nc.gpsimd if h == 0 else nc.vector
                    mul_eng.tensor_mul(
                        oT_sb[pair][qb : qb + 64, qc * 512 : (qc + 1) * 512],
                        oacc[0:64, :],
                        rep,
                    )
                # Output projection pieces: one task/slot in waves that start
                # after the qk splices end (slot 45) AND after this segment's
                # scale chain has landed (~8 slots past segment end --
                # chain latency is ~4-5 slots plus margin; pieces popped
                # earlier stall the in-order PE queue on the gpsimd mul).
                wave = max(45 + 8 * seg, seg * NT + NT + 8)
                for tb in range(4):
                    box = {"act_cast": wave + 2 * tb >= 126}
                    for dc in range(2):
                        enq(
                            wave + 2 * tb + dc,
                            outproj_piece(pair, qc * 4 + tb, dc, box),
                        )

        # Drain any tasks that remained (the last qc's output projection).
        while taskq:
            _, _, fn = heapq.heappop(taskq)
            fn()


def build():
    nc = bass.Bass("TRN2", target_bir_lowering=False)
    xT = nc.dram_tensor("xT", [D, N], bf16, kind="ExternalInput").ap()
    wqk = nc.dram_tensor("wqk", [D, 2 * HPG * HD], bf16, kind="ExternalInput").ap()
    wv = nc.dram_tensor("wv", [D, HPG * HD], bf16, kind="ExternalInput").ap()
    wo = nc.dram_tensor("wo", [2 * 128, D], bf16, kind="ExternalInput").ap()
    y0 = nc.dram_tensor("y0", [N, D], bf16, kind="ExternalOutput").ap()
    y1 = nc.dram_tensor("y1", [N, D], bf16, kind="ExternalOutput").ap()
    with _TC(nc) as tc:
        _body(nc, tc, xT, wqk, wv, wo, y0, y1)
    return nc


def shard_inputs(x, w_qkv, w_out):
    """Build the 8 per-core input maps from the full tensors (bf16)."""
    import ml_dtypes

    bfd = ml_dtypes.bfloat16
    x = np.asarray(x, dtype=np.float32)
    w_qkv = np.asarray(w_qkv, dtype=np.float32)
    w_out = np.asarray(w_out, dtype=np.float32)
    in_maps = []
    for c in range(NCORES):
        b, grp = c // 4, c % 4
        heads = [HPG * grp + i for i in range(HPG)]
        xTa = np.ascontiguousarray(x[b].T).astype(bfd)
        qcols = [w_qkv[:, h * HD : (h + 1) * HD] for h in heads]
        kcols = [w_qkv[:, H * HD + h * HD : H * HD + (h + 1) * HD] for h in heads]
        vcols = [w_qkv[:, 2 * H * HD + h * HD : 2 * H * HD + (h + 1) * HD] for h in heads]
        wqk_a = np.ascontiguousarray(np.concatenate(qcols + kcols, axis=1)).astype(bfd)
        wv_a = np.ascontiguousarray(np.concatenate(vcols, axis=1)).astype(bfd)
        wo_a = np.ascontiguousarray(
            np.concatenate([w_out[h * HD : (h + 1) * HD, :] for h in heads], axis=0)
        ).astype(bfd)
        in_maps.append({"xT": xTa, "wqk": wqk_a, "wv": wv_a, "wo": wo_a})
    return in_maps


LAST_RESULTS = None  # BassKernelResults from the most recent kernel() call
_NC_CACHE = None


def kernel(x, w_qkv, w_out):
    global LAST_RESULTS, _NC_CACHE
    if _NC_CACHE is None:
        _NC_CACHE = build()
    nc = _NC_CACHE
    in_maps = shard_inputs(x, w_qkv, w_out)
    trace = bool(os.environ.get("KERNEL_TRACE"))
    res = bass_utils.run_bass_kernel_spmd(
        nc, in_maps, core_ids=list(range(NCORES)), trace=trace
    )
    LAST_RESULTS = res
    y = np.zeros((B, N, D), dtype=np.float32)
    for c in range(NCORES):
        y[c // 4] += np.asarray(res.results[c]["y0"], dtype=np.float32)
        y[c // 4] += np.asarray(res.results[c]["y1"], dtype=np.float32)
    return y



# revision 38
# speedup vs baseline: 1.0015x; 1.0015x over previous
"""Multi-head attention (B=2, N=2048, D=1024, H=16, HD=64) on 8 TRN2 NeuronCores.

Sharding: core c handles batch b = c//4 and heads 4*(c%4) .. 4*(c%4)+3.
Each core computes the QKV projection for its 4 heads, attention, and a
partial output projection (contraction over its 256 hd-columns of w_out).
The host sums the per-core partials (y0 + y1 per core, 4 cores per batch).

V2 design (from trace analysis of the 283us baseline):
  - The kernel is two-engine-bound: Act (exp, ~142us floor) and PE
    (~136us streaming floor).  Everything else must hide under those.
  - Pair-OUTER loop (for pair: for qc:) so the pair-1 q/k projections
    have 64 kb-slots of PE slack to splice into, not 16.
  - All projection/output-projection PE work is diced into micro-tasks
    (<= ~0.45us each, the per-slot PE slack at the Act cadence) and
    popped one per kb-slot from a FIFO with eligibility times.
  - Dtypes: xT/wqk/wv/wo shipped bf16 (halves the input-DMA gate before
    the first exp); v/pt/oT bf16 (FWL weight loads for AV/outproj);
    qkT kept f32r (scores precision); y0/y1 partials bf16 (host sums f32).
  - Output projection split per head-pair into y0/y1 (single-matmul
    pieces, schedulable anywhere); host sums.
  - Every dma_start is a ~0.6us serial instruction on the SP engine, so
    trigger COUNT and ORDER are managed: segment-0's input quarters
    (qx-major xT layout) issue first, y pieces are one DMA each, and the
    softmax scale chain is 4 hops (den row -> DRAM -> [128,4] recip on
    DVE -> DRAM bf16 -> broadcast [64,512]), with the scale-mul on
    gpsimd.  Act does exp ONLY; DVE does all PSUM->SBUF copies.

Device-side layout per core:
  qkT/kT = w_qk.T @ x.T            [512, N]  (q/k per-head rows, f32r)
  v      = x @ w_v                 [N, 4*(HD+1)] bf16 (+ ones column per head)
  scoresT= kT.T-slices @ qT        [keys, queries] per head pair (row-split
           concurrent matmuls on PE row groups 0-63 / 64-127)
  pT     = exp(0.125 * scoresT)    bf16 (no max-sub: scores are O(few))
  oT|den = [v | 1].T @ pT          [65, queries] per head (row 64 = den)
  oT     = oT * (1/den)            bf16 (recip on DVE, mul on gpsimd)
  y{0,1} = oT.T-slices @ w_out     [N, D] partials per head pair
"""

import os
import sys
import types
import ctypes
import contextlib

import numpy as np
import bass_rust
import concourse.bass as bass
import concourse.tile as tile
from concourse import mybir
from concourse import bass_utils
from concourse import library_config
from concourse import masks
from concourse.vector_clock import ScopedClock


def _ensure_ntff_hook():
    """Provide antenv.axon_hooks if the container lacks it, so that
    run_bass_kernel_spmd(trace=True) works instead of raising."""
    if "antenv.axon_hooks" in sys.modules:
        return
    try:
        import antenv.axon_hooks  # noqa: F401

        return
    except ImportError:
        pass

    def _make_hook():
        so_path = "/opt/axon/libaxon_pjrt.so"
        try:
            lib = ctypes.CDLL(so_path)
        except OSError:
            return None
        if not hasattr(lib, "axon_start_nrt_profile"):
            return None
        lib.axon_start_nrt_profile.argtypes = [
            ctypes.POINTER(ctypes.c_int64),
            ctypes.c_size_t,
        ]
        lib.axon_start_nrt_profile.restype = ctypes.c_int64
        lib.axon_stop_nrt_profile.argtypes = [ctypes.c_char_p]
        lib.axon_stop_nrt_profile.restype = ctypes.c_int64

        @contextlib.contextmanager
        def _hook(output_dir, device_ids):
            import jax

            jax.devices()
            if device_ids:
                ids = (ctypes.c_int64 * len(device_ids))(*device_ids)
                rc = lib.axon_start_nrt_profile(ids, len(device_ids))
            else:
                rc = lib.axon_start_nrt_profile(None, 0)
            if rc != 0:
                raise RuntimeError(f"axon_start_nrt_profile rc={rc}")
            try:
                yield
            finally:
                lib.axon_stop_nrt_profile(str(output_dir).encode())

        return _hook

    hook = _make_hook()
    mod = types.ModuleType("antenv.axon_hooks")
    mod.get_axon_ntff_profile_hook = lambda: hook
    mod.set_axon_ntff_profile_hook = lambda h: None
    sys.modules["antenv.axon_hooks"] = mod


_ensure_ntff_hook()

B, N, D = 2, 2048, 1024
H, HD = 16, 64
HPG = 4  # heads per core
NCORES = 8
ND = D // 128  # 8 contraction chunks for the projections
NT = N // 128  # 16 token/key blocks
NQ = N // 512  # 4 query chunks

f32 = mybir.dt.float32
f32r = mybir.dt.float32r
bf16 = mybir.dt.bfloat16
EXP = mybir.ActivationFunctionType.Exp


class _TC(tile.TileContext):
    """TileContext adapted to this walrus build, which encodes at most ONE
    semaphore wait per instruction: excess waits are offloaded onto
    preceding same-engine nops, and the final drain is split the same way."""

    _ws_counter = 0

    def _lower_ordered_insts(self, ordered):
        for bbname, insts in ordered.items():
            new = []
            for inst in insts:
                si = inst.sync_info
                if (
                    si is not None
                    and len(si.on_wait) > 1
                    and inst.engine != mybir.EngineType.Unassigned
                ):
                    waits = list(si.on_wait)
                    ups = list(si.on_update)
                    for w in waits[:-1]:
                        _TC._ws_counter += 1
                        new.append(
                            mybir.InstNoOp(
                                name=f"waitsplit_{_TC._ws_counter}",
                                engine=inst.engine,
                                ins=[],
                                outs=[],
                                sync_info=bass_rust.SyncInfo(
                                    on_wait=[w], on_update=[]
                                ),
                                bass_nofuse=True,
                            )
                        )
                    inst.sync_info = bass_rust.SyncInfo(
                        on_wait=[waits[-1]], on_update=ups
                    )
                new.append(inst)
            ordered[bbname] = new
        super()._lower_ordered_insts(ordered)

    def _drain_and_barrier(self, tick_clock, wait_clock):
        nop0 = self.nc.sync.nop(nofuse=True)
        wait_clock.add_sem_waits(nop0.ins, ScopedClock({None: tick_clock.global_clock}))
        si = nop0.ins.sync_info
        waits = list(si.on_wait) if si is not None else []
        if len(waits) > 1:
            nop0.ins.sync_info = bass_rust.SyncInfo(on_wait=waits[:1], on_update=[])
            for i in range(1, len(waits)):
                n = self.nc.sync.nop(nofuse=True)
                n.ins.sync_info = bass_rust.SyncInfo(
                    on_wait=waits[i : i + 1], on_update=[]
                )
        self.nc.sync.drain()
        self.nc.all_engine_barrier()
        assert self.sems is not None
        popped = self.nc._tile_sem_poison_stack.pop()
        assert popped is self._sem_poison
        self.nc.clear_and_free_semaphores(list(self.sems.allocated().values()))
        self.nc.all_engine_barrier()


def _body(nc, tc, xT, wqk, wv, wo, y0, y1):
    with contextlib.ExitStack() as ctx:
        persist = ctx.enter_context(tc.tile_pool(name="persist", bufs=1))
        pt_pool = ctx.enter_context(tc.tile_pool(name="ptp", bufs=6))
        ysb_pool = ctx.enter_context(tc.tile_pool(name="ysbp", bufs=6))
        small = ctx.enter_context(tc.tile_pool(name="small", bufs=3))
        dscr = ctx.enter_context(tc.tile_pool(name="dscr", bufs=4, space="DRAM"))
        ps_s = ctx.enter_context(tc.tile_pool(name="ps_s", bufs=2, space="PSUM"))
        ps_o = ctx.enter_context(tc.tile_pool(name="ps_o", bufs=2, space="PSUM"))
        ps_mm = ctx.enter_context(tc.tile_pool(name="ps_mm", bufs=2, space="PSUM"))

        # ---- ACT table preload + PE clock warm-up ----
        # The exp table-set load (~1.3-2.7us) is inserted by walrus right
        # before the FIRST activation; a dummy exp on a scratch tile at t~0
        # hides the load under the input-DMA wait instead of paying it right
        # before exp(0).
        scrap = persist.tile([128, 8], f32, tag="scrap", name="scrap")
        scrap2 = persist.tile([128, 8], f32, tag="scrap2", name="scrap2")
        nc.vector.memset(scrap, 0.0)
        # The PE HAM clock gate defaults to 1.2 GHz and only releases to
        # 2.4 GHz after ~3.4us of sustained matmul activity.  The PE sits
        # idle for ~10us waiting on input DMA anyway, so run dummy matmuls
        # on scratch tiles to warm the clock before the first real matmul
        # (the V2 trace shows the whole prefix + early slots at half clock).
        scrapW = persist.tile([128, 128], bf16, tag="scrapW", name="scrapW")
        scrapX = persist.tile([128, 512], bf16, tag="scrapX", name="scrapX")
        nc.vector.memset(scrapW, 0.0)
        nc.vector.memset(scrapX, 0.0)
        # 16 dummies: ~6 run at the cold 1.2 GHz clock (~3.5us, warming the
        # HAM), the rest at 2.4 GHz keep it warm until the first input
        # chunks land (~13us) -- a >3.4us PE idle window would re-throttle
        # the clock and the whole prefix would run at half speed.
        for wi in range(16):
            ps_w = ps_mm.tile([128, 512], f32, tag="mm", name=f"ps_warm{wi}")
            nc.tensor.matmul(ps_w, lhsT=scrapW, rhs=scrapX, start=True, stop=True)

        # Constants for the PE-based softmax scale chain: a [128,1] ones
        # column (sliced at the needed base partition as a 1x1 "identity"
        # for the den-row transposes, and rows 0:1 x 64 as the broadcast
        # lhsT) and a [128,128] bf16 identity for the transpose back.
        onescol = persist.tile([128, 64], bf16, tag="onescol", name="onescol")
        nc.vector.memset(onescol, 1.0)
        onescol_f = persist.tile([128, 1], f32, tag="onescol_f", name="onescol_f")
        nc.vector.memset(onescol_f, 1.0)
        ident128 = persist.tile([128, 128], bf16, tag="ident128", name="ident128")
        masks.make_identity(nc, ident128)

        # ---- persistent SBUF residents + input DMA ----
        # Every dma_start is a ~0.6us SERIAL instruction on its issuing
        # engine's queue -- the issue ORDER and engine split matter as much
        # as the transfer itself.  The critical gate for exp(0) is
        # wqk (8 chunks) + xT quarter 0 (8 chunks): spread those 16 triggers
        # across FOUR engine queues (sync/vector/tensor/gpsimd) so they are
        # all in flight by ~2.5us.  Non-critical inputs follow on sync
        # (xT q1/q2), gpsimd (xT q3) and Act (wv, wo -- idle until exp(0)).
        xT_sb, wqk_sb, wv_sb = [], [], []
        for i in range(ND):
            wqk_sb.append(persist.tile([128, 2 * HPG * HD], bf16, tag=f"wqk{i}", name=f"wqk_sb{i}"))
            wv_sb.append(persist.tile([128, HPG * HD], bf16, tag=f"wv{i}", name=f"wv_sb{i}"))
            xT_sb.append(persist.tile([128, N], bf16, tag=f"xT{i}", name=f"xT_sb{i}"))
        # Only sync/scalar/gpsimd can trigger DMAs; split the 16 critical
        # transfers across all three queues.
        for i in range(6):
            nc.sync.dma_start(out=wqk_sb[i], in_=wqk[i * 128 : (i + 1) * 128, :])
        for i in range(6, ND):
            nc.scalar.dma_start(out=wqk_sb[i], in_=wqk[i * 128 : (i + 1) * 128, :])
        for i in range(6):
            nc.gpsimd.dma_start(
                out=xT_sb[i][:, 0:512], in_=xT[i * 128 : (i + 1) * 128, 0:512]
            )
        for i in range(6, ND):
            nc.scalar.dma_start(
                out=xT_sb[i][:, 0:512], in_=xT[i * 128 : (i + 1) * 128, 0:512]
            )
        # ACT table preload AFTER the critical triggers (walrus inserts the
        # ~1.3us table load right before this first ACTIVATE).
        nc.scalar.activation(scrap2, scrap, EXP)
        # Act: wv (needed by v(0) at ~slot 0) then wo (needed ~slot 45).
        for i in range(ND):
            nc.scalar.dma_start(out=wv_sb[i], in_=wv[i * 128 : (i + 1) * 128, :])
        wo_sb = []
        for c2 in range(2):
            t_ = persist.tile([128, D], bf16, tag=f"wo{c2}", name=f"wo_sb{c2}")
            nc.scalar.dma_start(out=t_, in_=wo[c2 * 128 : (c2 + 1) * 128, :])
            wo_sb.append(t_)
        # Sync: xT quarters 1-2 (needed from ~slot 0 resp. ~slot 3).
        for qx in range(1, 3):
            for i in range(ND):
                nc.sync.dma_start(
                    out=xT_sb[i][:, qx * 512 : (qx + 1) * 512],
                    in_=xT[i * 128 : (i + 1) * 128, qx * 512 : (qx + 1) * 512],
                )

        # qkT rows: tile 0 = qT heads 0,1 | tile 1 = qT heads 2,3
        #           tile 2 = kT heads 0,1 | tile 3 = kT heads 2,3
        # bf16: the scores matmuls stream 2x faster than f32r (saves ~29us
        # of PE), at ~0.3% extra noise on p (well inside the 2e-2 gate).
        qkT_sb = [
            persist.tile([128, N], bf16, tag=f"qkT{r}", name=f"qkT_sb{r}")
            for r in range(4)
        ]
        # v blocks with a ones column after each head: [v_h | 1] x 4
        v_sb = [
            persist.tile([128, HPG * (HD + 1)], bf16, tag=f"v{t}", name=f"v_sb{t}")
            for t in range(NT)
        ]
        oT_sb = [
            persist.tile([128, N], bf16, tag=f"oT{c2}", name=f"oT_sb{c2}")
            for c2 in range(2)
        ]
        # gpsimd queue: ones-memsets for the earliest v blocks first (v(0)
        # is consumed at slot 0), then the non-critical xT quarter 3
        # triggers (needed from ~slot 7), then the remaining memsets and
        # the ucode library for partition_broadcast + tensor_tensor divide
        # (needed from ~slot 16, the first scale chain).
        for t in range(4):
            nc.gpsimd.memset(v_sb[t], 1.0)
        for i in range(ND):
            nc.gpsimd.dma_start(
                out=xT_sb[i][:, 1536:2048],
                in_=xT[i * 128 : (i + 1) * 128, 1536:2048],
            )
        for t in range(4, NT):
            nc.gpsimd.memset(v_sb[t], 1.0)

        # ---- projection group emitters ----
        # Each emitter yields micro-steps of <= ~0.45us of PE work; the
        # final step does the PSUM->SBUF copy (on DVE).
        pools3 = (ps_mm, ps_o, ps_s)
        tags3 = ("mm", "o", "s")
        group_idx = 0

        def qk_group_steps(r, qc, force_mm=False):
            """4 matmul steps (2 accum chunks each) + 1 copy step."""
            nonlocal group_idx
            sel = 0 if force_mm else group_idx % 3
            group_idx += 1
            pool, tag = pools3[sel], tags3[sel]
            box = {}

            def mmstep(i0):
                if i0 == 0:
                    box["ps"] = pool.tile([128, 512], f32, tag=tag, name=f"ps_qk_{r}_{qc}")
                for i in range(i0, i0 + 2):
                    nc.tensor.matmul(
                        box["ps"],
                        lhsT=wqk_sb[i][:, r * 128 : (r + 1) * 128],
                        rhs=xT_sb[i][:, qc * 512 : (qc + 1) * 512],
                        start=(i == 0),
                        stop=(i == ND - 1),
                    )

            def cpstep():
                nc.vector.tensor_copy(
                    qkT_sb[r][:, qc * 512 : (qc + 1) * 512], box["ps"]
                )

            return [lambda i0=i0: mmstep(i0) for i0 in range(0, ND, 2)] + [cpstep]

        def v_group_steps(t, force_mm=False):
            """2 matmul steps (4 accum chunks each) + 1 copy step."""
            nonlocal group_idx
            sel = 0 if force_mm else group_idx % 3
            group_idx += 1
            pool, tag = pools3[sel], tags3[sel]
            box = {}

            def mmstep(i0):
                if i0 == 0:
                    box["ps"] = pool.tile([128, HPG * HD], f32, tag=tag, name=f"ps_v_{t}")
                for i in range(i0, i0 + 4):
                    nc.tensor.matmul(
                        box["ps"],
                        lhsT=xT_sb[i][:, t * 128 : (t + 1) * 128],
                        rhs=wv_sb[i],
                        start=(i == 0),
                        stop=(i == ND - 1),
                    )

            def cpstep():
                vview = v_sb[t].rearrange("p (h c) -> p h c", c=HD + 1)[:, :, 0:HD]
                nc.vector.tensor_copy(vview, box["ps"].rearrange("p (h c) -> p h c", c=HD))

            return [lambda i0=i0: mmstep(i0) for i0 in range(0, ND, 4)] + [cpstep]

        def outproj_piece(c2, t, dc, box):
            """One micro-task: single matmul + copy; the dc=1 task DMAs the
            full [128, 1024] row-block in ONE transfer (the two dc halves
            share an SBUF staging tile), halving y trigger count to 32.
            Triggers alternate sync/gpsimd rings (the scale chain no longer
            uses DMA, so both rings are safe).  Drain-wave pieces CAST via
            the Act engine (idle once the exps are done) so the tail is not
            serialized on DVE."""
            ydst = y0 if c2 == 0 else y1
            trig = nc.sync if t % 2 == 0 else nc.gpsimd

            def step():
                ps = ps_mm.tile([128, 512], f32, tag="mm", name=f"ps_y{c2}_{t}_{dc}")
                nc.tensor.matmul(
                    ps,
                    lhsT=oT_sb[c2][:, t * 128 : (t + 1) * 128],
                    rhs=wo_sb[c2][:, dc * 512 : (dc + 1) * 512],
                    start=True,
                    stop=True,
                )
                if dc == 0:
                    box["ysb"] = ysb_pool.tile(
                        [128, 1024], bf16, tag="y", name=f"ysb{c2}_{t}"
                    )
                ysb = box["ysb"]
                if box.get("act_cast"):
                    nc.scalar.copy(ysb[:, dc * 512 : (dc + 1) * 512], ps)
                else:
                    nc.vector.tensor_copy(ysb[:, dc * 512 : (dc + 1) * 512], ps)
                if dc == 1:
                    trig.dma_start(
                        out=ydst[t * 128 : (t + 1) * 128, :],
                        in_=ysb,
                    )

            return [step]

        # ---- prefix: only what slot 0's scores need ----
        # qT qc0 + kT kb0-3, gated on xT quarter 0 + wqk.  The two groups
        # are interleaved PER CHUNK so the last-arriving input chunk gates
        # only the final MM pair, not a whole second 8-chunk pass.
        # Everything else (including v(0..3)) is spliced into the slots.
        def run_all(steps):
            for s in steps:
                s()

        psQ = ps_mm.tile([128, 512], f32, tag="mm", name="ps_qk_pre0")
        psK = ps_o.tile([128, 512], f32, tag="o", name="ps_qk_pre2")
        for i in range(ND):
            nc.tensor.matmul(
                psQ,
                lhsT=wqk_sb[i][:, 0:128],
                rhs=xT_sb[i][:, 0:512],
                start=(i == 0),
                stop=(i == ND - 1),
            )
            nc.tensor.matmul(
                psK,
                lhsT=wqk_sb[i][:, 2 * 128 : 3 * 128],
                rhs=xT_sb[i][:, 0:512],
                start=(i == 0),
                stop=(i == ND - 1),
            )
        nc.vector.tensor_copy(qkT_sb[0][:, 0:512], psQ)
        nc.vector.tensor_copy(qkT_sb[2][:, 0:512], psK)

        # ---- micro-task queue for the kb-slot loop ----
        # Tasks become eligible at a given global slot; each slot pops ALL
        # eligible tasks in eligibility order (min-heap -- a far-future
        # task must not block an overdue one behind it).
        import heapq

        taskq = []  # heap of (eligible_slot, seq, fn)
        seq_ctr = [0]

        def enq(slot, steps):
            for k, s in enumerate(steps):
                heapq.heappush(taskq, (slot + k, seq_ctr[0], s))
                seq_ctr[0] += 1

        # v blocks during early pair-0 slots.  av_pair(t) is emitted in slot
        # t+1 and pops happen before it, so v(t)'s 3 steps must be popped
        # by slot t (eligibility t-2 with pop-all-eligible).  kT qc2/qc3
        # are READ by the scores emitted at slots 8/12, which precede that
        # slot's pop -- their 5 tasks must pop by slot 4qc-1.  The early
        # slots are PE-production-bound regardless; multi-pop is correct.
        for t in range(NT):
            enq(t - 2, v_group_steps(t, force_mm=True))
        enq(-1, qk_group_steps(2, 1, force_mm=True))      # kT pair0 kb 4-7 (by slot 3)
        enq(3, qk_group_steps(2, 2, force_mm=True))       # kT pair0 kb 8-11
        enq(7, qk_group_steps(2, 3, force_mm=True))       # kT pair0 kb 12-15
        enq(11, qk_group_steps(0, 1, force_mm=True))      # qT pair0 qc1 (slot 16)
        # Pair-1 projections and qT pair0 qc2/3: confined to slots 16-44 so
        # they never share the ps_mm ring with the output-projection pieces
        # (waves start at slot 45) -- a spliced group holds its PSUM buf for
        # ~5 slots and stalls the 2-buf ring if pieces interleave.
        enq(16, qk_group_steps(0, 2, force_mm=True))      # qT pair0 qc2 (slot 32)
        enq(19, qk_group_steps(2 + 1, 0, force_mm=True))  # kT pair1
        enq(22, qk_group_steps(2 + 1, 1, force_mm=True))
        enq(25, qk_group_steps(2 + 1, 2, force_mm=True))
        enq(28, qk_group_steps(2 + 1, 3, force_mm=True))
        enq(31, qk_group_steps(0, 3, force_mm=True))      # qT pair0 qc3 (slot 48)
        enq(34, qk_group_steps(1, 0, force_mm=True))      # qT pair1
        enq(37, qk_group_steps(1, 1, force_mm=True))
        enq(40, qk_group_steps(1, 2, force_mm=True))
        enq(43, qk_group_steps(1, 3, force_mm=True))

        def pop_task(g):
            popped = False
            while taskq and taskq[0][0] <= g:
                _, _, fn = heapq.heappop(taskq)
                fn()
                popped = True
            return popped

        # ---- phase 2: attention, pair-outer ----
        def av_pair(pair, poA, poB, kb, pt):
            hA, hB = 2 * pair, 2 * pair + 1
            nc.tensor.matmul(
                poA,
                lhsT=v_sb[kb][:, hA * (HD + 1) : (hA + 1) * (HD + 1)],
                rhs=pt[:, 0:512],
                start=(kb == 0),
                stop=(kb == NT - 1),
            )
            nc.tensor.matmul(
                poB,
                lhsT=v_sb[kb][:, hB * (HD + 1) : (hB + 1) * (HD + 1)],
                rhs=pt[:, 512:1024],
                start=(kb == 0),
                stop=(kb == NT - 1),
            )

        for pair in range(2):
            for qc in range(NQ):
                seg = pair * NQ + qc
                poA = ps_o.tile([65, 512], f32, tag="o", name=f"poA_{pair}_{qc}")
                poB = ps_o.tile([65, 512], f32, tag="o", name=f"poB_{pair}_{qc}")
                pending = None
                for kb in range(NT):
                    g = seg * NT + kb
                    ps = ps_s.tile(
                        [128, 1024], f32, tag="s", name=f"ps_s_{pair}_{qc}_{kb}"
                    )
                    # Row-split concurrent pair: head A on PE rows 0-63,
                    # head B on rows 64-127.
                    nc.tensor.matmul(
                        ps[:, 0:512],
                        lhsT=qkT_sb[2 + pair][0:64, kb * 128 : (kb + 1) * 128],
                        rhs=qkT_sb[pair][0:64, qc * 512 : (qc + 1) * 512],
                        start=True,
                        stop=True,
                    )
                    nc.tensor.matmul(
                        ps[:, 512:1024],
                        lhsT=qkT_sb[2 + pair][64:128, kb * 128 : (kb + 1) * 128],
                        rhs=qkT_sb[pair][64:128, qc * 512 : (qc + 1) * 512],
                        start=True,
                        stop=True,
                    )
                    pt = pt_pool.tile(
                        [128, 1024], bf16, tag="pt", name=f"pt_{pair}_{qc}_{kb}"
                    )
                    nc.scalar.activation(pt, ps, EXP, scale=HD**-0.5)
                    pop_task(g)
                    if pending is not None:
                        av_pair(pair, poA, poB, *pending)
                    pending = (kb, pt)
                av_pair(pair, poA, poB, *pending)

                # Softmax scale chain, fully ON-CHIP (no DMA, which queued
                # ~5us/hop behind in-flight y pieces; no custom ISA, which
                # this walrus build cannot codegen):
                #   1. den row [1,512] -> [128,4] via 4 tiny PE transposes
                #      (the PE is right here at the segment end anyway)
                #   2. [128,4] reciprocal on DVE (~0.2us, vs 3.3us for the
                #      single-lane row reciprocal)
                #   3. transpose back to a [1,512] row via 4 PE transposes
                #   4. 64-partition broadcast = rank-1 matmul ones64.T @ row
                #   5. scale-mul on gpsimd
                for h, po in ((0, poA), (1, poB)):
                    qb = h * 64
                    oacc = small.tile(
                        [65, 512], f32, tag="oacc", name=f"oacc_{seg}_{h}", bufs=4
                    )
                    nc.vector.tensor_copy(oacc, po)
                    pden = ps_mm.tile([128, 4], f32, tag="mm", name=f"pden_{seg}_{h}")
                    for j in range(4):
                        nc.tensor.transpose(
                            pden[:, j : j + 1],
                            oacc[64:65, 128 * j : 128 * (j + 1)],
                            onescol_f[64:65, 0:1],
                            tile_position=(64, 0),
                        )
                    rout = small.tile([128, 4], bf16, tag="rout", name=f"rout_{seg}_{h}")
                    with nc.allow_low_precision("softmax denom recip, bf16 rep"):
                        nc.vector.reciprocal(rout, pden)
                    rrow_ps = ps_mm.tile([1, 512], bf16, tag="mm", name=f"rrowp_{seg}_{h}")
                    for j in range(4):
                        nc.tensor.transpose(
                            rrow_ps[0:1, 128 * j : 128 * (j + 1)],
                            rout[:, j : j + 1],
                            ident128,
                        )
                    rrow = small.tile([1, 512], bf16, tag="rrow", name=f"rrow_{seg}_{h}")
                    nc.vector.tensor_copy(rrow, rrow_ps)
                    rep_ps = ps_mm.tile([64, 512], f32, tag="mm", name=f"repp_{seg}_{h}")
                    nc.tensor.matmul(
                        rep_ps, lhsT=onescol[0:1, :], rhs=rrow, start=True, stop=True
                    )
                    rep = small.tile([64, 512], bf16, tag="rep", name=f"rep_{seg}_{h}")
                    nc.vector.tensor_copy(rep, rep_ps)
                    # h0's mul on gpsimd, h1's on DVE: the two run
                    # concurrently instead of serializing on gpsimd.
                    mul_eng = nc.gpsimd if h == 0 else nc.vector
                    mul_eng.tensor_mul(
                        oT_sb[pair][qb : qb + 64, qc * 512 : (qc + 1) * 512],
                        oacc[0:64, :],
                        rep,
                    )
                # Output projection pieces: one task/slot in waves that start
                # after the qk splices end (slot 45) AND after this segment's
                # scale chain has landed (~8 slots past segment end --
                # chain latency is ~4-5 slots plus margin; pieces popped
                # earlier stall the in-order PE queue on the gpsimd mul).
                wave = max(45 + 8 * seg, seg * NT + NT + 8)
                for tb in range(4):
                    box = {"act_cast": wave + 2 * tb >= 126}
                    for dc in range(2):
                        enq(
                            wave + 2 * tb + dc,
                            outproj_piece(pair, qc * 4 + tb, dc, box),
                        )

        # Drain any tasks that remained (the last qc's output projection).
        while taskq:
            _, _, fn = heapq.heappop(taskq)
            fn()


def build():
    nc = bass.Bass("TRN2", target_bir_lowering=False)
    xT = nc.dram_tensor("xT", [D, N], bf16, kind="ExternalInput").ap()
    wqk = nc.dram_tensor("wqk", [D, 2 * HPG * HD], bf16, kind="ExternalInput").ap()
    wv = nc.dram_tensor("wv", [D, HPG * HD], bf16, kind="ExternalInput").ap()
    wo = nc.dram_tensor("wo", [2 * 128, D], bf16, kind="ExternalInput").ap()
    y0 = nc.dram_tensor("y0", [N, D], bf16, kind="ExternalOutput").ap()
    y1 = nc.dram_tensor("y1", [N, D], bf16, kind="ExternalOutput").ap()
    with _TC(nc) as tc:
        _body(nc, tc, xT, wqk, wv, wo, y0, y1)
    return nc


def shard_inputs(x, w_qkv, w_out):
    """Build the 8 per-core input maps from the full tensors (bf16)."""
    import ml_dtypes

    bfd = ml_dtypes.bfloat16
    x = np.asarray(x, dtype=np.float32)
    w_qkv = np.asarray(w_qkv, dtype=np.float32)
    w_out = np.asarray(w_out, dtype=np.float32)
    in_maps = []
    for c in range(NCORES):
        b, grp = c // 4, c % 4
        heads = [HPG * grp + i for i in range(HPG)]
        xTa = np.ascontiguousarray(x[b].T).astype(bfd)
        qcols = [w_qkv[:, h * HD : (h + 1) * HD] for h in heads]
        kcols = [w_qkv[:, H * HD + h * HD : H * HD + (h + 1) * HD] for h in heads]
        vcols = [w_qkv[:, 2 * H * HD + h * HD : 2 * H * HD + (h + 1) * HD] for h in heads]
        wqk_a = np.ascontiguousarray(np.concatenate(qcols + kcols, axis=1)).astype(bfd)
        wv_a = np.ascontiguousarray(np.concatenate(vcols, axis=1)).astype(bfd)
        wo_a = np.ascontiguousarray(
            np.concatenate([w_out[h * HD : (h + 1) * HD, :] for h in heads], axis=0)
        ).astype(bfd)
        in_maps.append({"xT": xTa, "wqk": wqk_a, "wv": wv_a, "wo": wo_a})
    return in_maps


LAST_RESULTS = None  # BassKernelResults from the most recent kernel() call
_NC_CACHE = None


def kernel(x, w_qkv, w_out):
    global LAST_RESULTS, _NC_CACHE
    if _NC_CACHE is None:
        _NC_CACHE = build()
    nc = _NC_CACHE
    in_maps = shard_inputs(x, w_qkv, w_out)
    trace = bool(os.environ.get("KERNEL_TRACE"))
    res = bass_utils.run_bass_kernel_spmd(
        nc, in_maps, core_ids=list(range(NCORES)), trace=trace
    )
    LAST_RESULTS = res
    y = np.zeros((B, N, D), dtype=np.float32)
    for c in range(NCORES):
        y[c // 4] += np.asarray(res.results[c]["y0"], dtype=np.float32)
        y[c // 4] += np.asarray(res.results[c]["y1"], dtype=np.float32)
    return y



# revision 39
# speedup vs baseline: 1.0245x; 1.0230x over previous
"""Multi-head attention (B=2, N=2048, D=1024, H=16, HD=64) on 8 TRN2 NeuronCores.

Sharding: core c handles batch b = c//4 and heads 4*(c%4) .. 4*(c%4)+3.
Each core computes the QKV projection for its 4 heads, attention, and a
partial output projection (contraction over its 256 hd-columns of w_out).
The host sums the per-core partials (y0 + y1 per core, 4 cores per batch).

V2 design (from trace analysis of the 283us baseline):
  - The kernel is two-engine-bound: Act (exp, ~142us floor) and PE
    (~136us streaming floor).  Everything else must hide under those.
  - Pair-OUTER loop (for pair: for qc:) so the pair-1 q/k projections
    have 64 kb-slots of PE slack to splice into, not 16.
  - All projection/output-projection PE work is diced into micro-tasks
    (<= ~0.45us each, the per-slot PE slack at the Act cadence) and
    popped one per kb-slot from a FIFO with eligibility times.
  - Dtypes: xT/wqk/wv/wo shipped bf16 (halves the input-DMA gate before
    the first exp); v/pt/oT bf16 (FWL weight loads for AV/outproj);
    qkT kept f32r (scores precision); y0/y1 partials bf16 (host sums f32).
  - Output projection split per head-pair into y0/y1 (single-matmul
    pieces, schedulable anywhere); host sums.
  - Every dma_start is a ~0.6us serial instruction on the SP engine, so
    trigger COUNT and ORDER are managed: segment-0's input quarters
    (qx-major xT layout) issue first, y pieces are one DMA each, and the
    softmax scale chain is 4 hops (den row -> DRAM -> [128,4] recip on
    DVE -> DRAM bf16 -> broadcast [64,512]), with the scale-mul on
    gpsimd.  Act does exp ONLY; DVE does all PSUM->SBUF copies.

Device-side layout per core:
  qkT/kT = w_qk.T @ x.T            [512, N]  (q/k per-head rows, f32r)
  v      = x @ w_v                 [N, 4*(HD+1)] bf16 (+ ones column per head)
  scoresT= kT.T-slices @ qT        [keys, queries] per head pair (row-split
           concurrent matmuls on PE row groups 0-63 / 64-127)
  pT     = exp(0.125 * scoresT)    bf16 (no max-sub: scores are O(few))
  oT|den = [v | 1].T @ pT          [65, queries] per head (row 64 = den)
  oT     = oT * (1/den)            bf16 (recip on DVE, mul on gpsimd)
  y{0,1} = oT.T-slices @ w_out     [N, D] partials per head pair
"""

import os
import sys
import types
import ctypes
import contextlib

import numpy as np
import bass_rust
import concourse.bass as bass
import concourse.tile as tile
from concourse import mybir
from concourse import bass_utils
from concourse import library_config
from concourse import masks
from concourse.vector_clock import ScopedClock


def _ensure_ntff_hook():
    """Provide antenv.axon_hooks if the container lacks it, so that
    run_bass_kernel_spmd(trace=True) works instead of raising."""
    if "antenv.axon_hooks" in sys.modules:
        return
    try:
        import antenv.axon_hooks  # noqa: F401

        return
    except ImportError:
        pass

    def _make_hook():
        so_path = "/opt/axon/libaxon_pjrt.so"
        try:
            lib = ctypes.CDLL(so_path)
        except OSError:
            return None
        if not hasattr(lib, "axon_start_nrt_profile"):
            return None
        lib.axon_start_nrt_profile.argtypes = [
            ctypes.POINTER(ctypes.c_int64),
            ctypes.c_size_t,
        ]
        lib.axon_start_nrt_profile.restype = ctypes.c_int64
        lib.axon_stop_nrt_profile.argtypes = [ctypes.c_char_p]
        lib.axon_stop_nrt_profile.restype = ctypes.c_int64

        @contextlib.contextmanager
        def _hook(output_dir, device_ids):
            import jax

            jax.devices()
            if device_ids:
                ids = (ctypes.c_int64 * len(device_ids))(*device_ids)
                rc = lib.axon_start_nrt_profile(ids, len(device_ids))
            else:
                rc = lib.axon_start_nrt_profile(None, 0)
            if rc != 0:
                raise RuntimeError(f"axon_start_nrt_profile rc={rc}")
            try:
                yield
            finally:
                lib.axon_stop_nrt_profile(str(output_dir).encode())

        return _hook

    hook = _make_hook()
    mod = types.ModuleType("antenv.axon_hooks")
    mod.get_axon_ntff_profile_hook = lambda: hook
    mod.set_axon_ntff_profile_hook = lambda h: None
    sys.modules["antenv.axon_hooks"] = mod


_ensure_ntff_hook()

B, N, D = 2, 2048, 1024
H, HD = 16, 64
HPG = 4  # heads per core
NCORES = 8
ND = D // 128  # 8 contraction chunks for the projections
NT = N // 128  # 16 token/key blocks
NQ = N // 512  # 4 query chunks

f32 = mybir.dt.float32
f32r = mybir.dt.float32r
bf16 = mybir.dt.bfloat16
EXP = mybir.ActivationFunctionType.Exp


class _TC(tile.TileContext):
    """TileContext adapted to this walrus build, which encodes at most ONE
    semaphore wait per instruction: excess waits are offloaded onto
    preceding same-engine nops, and the final drain is split the same way."""

    _ws_counter = 0

    def _lower_ordered_insts(self, ordered):
        for bbname, insts in ordered.items():
            new = []
            for inst in insts:
                si = inst.sync_info
                if (
                    si is not None
                    and len(si.on_wait) > 1
                    and inst.engine != mybir.EngineType.Unassigned
                ):
                    waits = list(si.on_wait)
                    ups = list(si.on_update)
                    for w in waits[:-1]:
                        _TC._ws_counter += 1
                        new.append(
                            mybir.InstNoOp(
                                name=f"waitsplit_{_TC._ws_counter}",
                                engine=inst.engine,
                                ins=[],
                                outs=[],
                                sync_info=bass_rust.SyncInfo(
                                    on_wait=[w], on_update=[]
                                ),
                                bass_nofuse=True,
                            )
                        )
                    inst.sync_info = bass_rust.SyncInfo(
                        on_wait=[waits[-1]], on_update=ups
                    )
                new.append(inst)
            ordered[bbname] = new
        super()._lower_ordered_insts(ordered)

    def _drain_and_barrier(self, tick_clock, wait_clock):
        nop0 = self.nc.sync.nop(nofuse=True)
        wait_clock.add_sem_waits(nop0.ins, ScopedClock({None: tick_clock.global_clock}))
        si = nop0.ins.sync_info
        waits = list(si.on_wait) if si is not None else []
        if len(waits) > 1:
            nop0.ins.sync_info = bass_rust.SyncInfo(on_wait=waits[:1], on_update=[])
            for i in range(1, len(waits)):
                n = self.nc.sync.nop(nofuse=True)
                n.ins.sync_info = bass_rust.SyncInfo(
                    on_wait=waits[i : i + 1], on_update=[]
                )
        self.nc.sync.drain()
        self.nc.all_engine_barrier()
        assert self.sems is not None
        popped = self.nc._tile_sem_poison_stack.pop()
        assert popped is self._sem_poison
        self.nc.clear_and_free_semaphores(list(self.sems.allocated().values()))
        self.nc.all_engine_barrier()


def _body(nc, tc, xT, wqk, wv, wo, y0, y1):
    with contextlib.ExitStack() as ctx:
        persist = ctx.enter_context(tc.tile_pool(name="persist", bufs=1))
        pt_pool = ctx.enter_context(tc.tile_pool(name="ptp", bufs=6))
        ysb_pool = ctx.enter_context(tc.tile_pool(name="ysbp", bufs=6))
        small = ctx.enter_context(tc.tile_pool(name="small", bufs=3))
        dscr = ctx.enter_context(tc.tile_pool(name="dscr", bufs=4, space="DRAM"))
        ps_s = ctx.enter_context(tc.tile_pool(name="ps_s", bufs=2, space="PSUM"))
        ps_o = ctx.enter_context(tc.tile_pool(name="ps_o", bufs=2, space="PSUM"))
        ps_mm = ctx.enter_context(tc.tile_pool(name="ps_mm", bufs=2, space="PSUM"))

        # ---- ACT table preload + PE clock warm-up ----
        # The exp table-set load (~1.3-2.7us) is inserted by walrus right
        # before the FIRST activation; a dummy exp on a scratch tile at t~0
        # hides the load under the input-DMA wait instead of paying it right
        # before exp(0).
        scrap = persist.tile([128, 8], f32, tag="scrap", name="scrap")
        scrap2 = persist.tile([128, 8], f32, tag="scrap2", name="scrap2")
        nc.vector.memset(scrap, 0.0)
        # The PE HAM clock gate defaults to 1.2 GHz and only releases to
        # 2.4 GHz after ~3.4us of sustained matmul activity.  The PE sits
        # idle for ~10us waiting on input DMA anyway, so run dummy matmuls
        # on scratch tiles to warm the clock before the first real matmul
        # (the V2 trace shows the whole prefix + early slots at half clock).
        scrapW = persist.tile([128, 128], bf16, tag="scrapW", name="scrapW")
        scrapX = persist.tile([128, 512], bf16, tag="scrapX", name="scrapX")
        nc.vector.memset(scrapW, 0.0)
        nc.vector.memset(scrapX, 0.0)
        # 16 dummies: ~6 run at the cold 1.2 GHz clock (~3.5us, warming the
        # HAM), the rest at 2.4 GHz keep it warm until the first input
        # chunks land (~13us) -- a >3.4us PE idle window would re-throttle
        # the clock and the whole prefix would run at half speed.
        for wi in range(16):
            ps_w = ps_mm.tile([128, 512], f32, tag="mm", name=f"ps_warm{wi}")
            nc.tensor.matmul(ps_w, lhsT=scrapW, rhs=scrapX, start=True, stop=True)

        # Constants for the PE-based softmax scale chain: a [128,1] ones
        # column (sliced at the needed base partition as a 1x1 "identity"
        # for the den-row transposes, and rows 0:1 x 64 as the broadcast
        # lhsT) and a [128,128] bf16 identity for the transpose back.
        onescol = persist.tile([128, 64], bf16, tag="onescol", name="onescol")
        nc.vector.memset(onescol, 1.0)
        onescol_f = persist.tile([128, 1], f32, tag="onescol_f", name="onescol_f")
        nc.vector.memset(onescol_f, 1.0)
        ident128 = persist.tile([128, 128], bf16, tag="ident128", name="ident128")
        masks.make_identity(nc, ident128)

        # ---- persistent SBUF residents + input DMA ----
        # Every dma_start is a ~0.6us SERIAL instruction on its issuing
        # engine's queue -- the issue ORDER and engine split matter as much
        # as the transfer itself.  The critical gate for exp(0) is
        # wqk (8 chunks) + xT quarter 0 (8 chunks): spread those 16 triggers
        # across FOUR engine queues (sync/vector/tensor/gpsimd) so they are
        # all in flight by ~2.5us.  Non-critical inputs follow on sync
        # (xT q1/q2), gpsimd (xT q3) and Act (wv, wo -- idle until exp(0)).
        xT_sb, wqk_sb, wv_sb = [], [], []
        for i in range(ND):
            wqk_sb.append(persist.tile([128, 2 * HPG * HD], bf16, tag=f"wqk{i}", name=f"wqk_sb{i}"))
            wv_sb.append(persist.tile([128, HPG * HD], bf16, tag=f"wv{i}", name=f"wv_sb{i}"))
            xT_sb.append(persist.tile([128, N], bf16, tag=f"xT{i}", name=f"xT_sb{i}"))
        # Only sync/scalar/gpsimd can trigger DMAs; split the 16 critical
        # transfers across all three queues.
        # Critical 16 (wqk + xT q0) split 5/5/6; then wv right behind on
        # all three queues (the scheduler hoists the v(0) matmuls ahead of
        # scores(0) in the PE queue, so LATE wv arrivals gate exp(0) --
        # measured +8us when wv rode behind the dummy-exp table load).
        for i in range(5):
            nc.sync.dma_start(out=wqk_sb[i], in_=wqk[i * 128 : (i + 1) * 128, :])
        for i in range(5, ND):
            nc.scalar.dma_start(out=wqk_sb[i], in_=wqk[i * 128 : (i + 1) * 128, :])
        for i in range(6):
            nc.gpsimd.dma_start(
                out=xT_sb[i][:, 0:512], in_=xT[i * 128 : (i + 1) * 128, 0:512]
            )
        for i in range(6, ND):
            nc.scalar.dma_start(
                out=xT_sb[i][:, 0:512], in_=xT[i * 128 : (i + 1) * 128, 0:512]
            )
        # ACT table preload AFTER the critical triggers (walrus inserts the
        # ~1.3us table load right before this first ACTIVATE).
        nc.scalar.activation(scrap2, scrap, EXP)
        for i in range(3):
            nc.sync.dma_start(out=wv_sb[i], in_=wv[i * 128 : (i + 1) * 128, :])
        for i in range(3, 6):
            nc.scalar.dma_start(out=wv_sb[i], in_=wv[i * 128 : (i + 1) * 128, :])
        for i in range(6, ND):
            nc.gpsimd.dma_start(out=wv_sb[i], in_=wv[i * 128 : (i + 1) * 128, :])
        wo_sb = []
        for c2 in range(2):
            t_ = persist.tile([128, D], bf16, tag=f"wo{c2}", name=f"wo_sb{c2}")
            nc.scalar.dma_start(out=t_, in_=wo[c2 * 128 : (c2 + 1) * 128, :])
            wo_sb.append(t_)
        # Sync: xT quarters 1-2 (needed from ~slot 0 resp. ~slot 3).
        for qx in range(1, 3):
            for i in range(ND):
                nc.sync.dma_start(
                    out=xT_sb[i][:, qx * 512 : (qx + 1) * 512],
                    in_=xT[i * 128 : (i + 1) * 128, qx * 512 : (qx + 1) * 512],
                )

        # qkT rows: tile 0 = qT heads 0,1 | tile 1 = qT heads 2,3
        #           tile 2 = kT heads 0,1 | tile 3 = kT heads 2,3
        # bf16: the scores matmuls stream 2x faster than f32r (saves ~29us
        # of PE), at ~0.3% extra noise on p (well inside the 2e-2 gate).
        qkT_sb = [
            persist.tile([128, N], bf16, tag=f"qkT{r}", name=f"qkT_sb{r}")
            for r in range(4)
        ]
        # v blocks with a ones column after each head: [v_h | 1] x 4
        v_sb = [
            persist.tile([128, HPG * (HD + 1)], bf16, tag=f"v{t}", name=f"v_sb{t}")
            for t in range(NT)
        ]
        oT_sb = [
            persist.tile([128, N], bf16, tag=f"oT{c2}", name=f"oT_sb{c2}")
            for c2 in range(2)
        ]
        # gpsimd queue: ones-memsets for the earliest v blocks first (v(0)
        # is consumed at slot 0), then the non-critical xT quarter 3
        # triggers (needed from ~slot 7), then the remaining memsets and
        # the ucode library for partition_broadcast + tensor_tensor divide
        # (needed from ~slot 16, the first scale chain).
        for t in range(4):
            nc.gpsimd.memset(v_sb[t], 1.0)
        for i in range(ND):
            nc.gpsimd.dma_start(
                out=xT_sb[i][:, 1536:2048],
                in_=xT[i * 128 : (i + 1) * 128, 1536:2048],
            )
        for t in range(4, NT):
            nc.gpsimd.memset(v_sb[t], 1.0)

        # ---- projection group emitters ----
        # Each emitter yields micro-steps of <= ~0.45us of PE work; the
        # final step does the PSUM->SBUF copy (on DVE).
        pools3 = (ps_mm, ps_o, ps_s)
        tags3 = ("mm", "o", "s")
        group_idx = 0

        def qk_group_steps(r, qc, force_mm=False):
            """4 matmul steps (2 accum chunks each) + 1 copy step."""
            nonlocal group_idx
            sel = 0 if force_mm else group_idx % 3
            group_idx += 1
            pool, tag = pools3[sel], tags3[sel]
            box = {}

            def mmstep(i0):
                if i0 == 0:
                    box["ps"] = pool.tile([128, 512], f32, tag=tag, name=f"ps_qk_{r}_{qc}")
                for i in range(i0, i0 + 2):
                    nc.tensor.matmul(
                        box["ps"],
                        lhsT=wqk_sb[i][:, r * 128 : (r + 1) * 128],
                        rhs=xT_sb[i][:, qc * 512 : (qc + 1) * 512],
                        start=(i == 0),
                        stop=(i == ND - 1),
                    )

            def cpstep():
                nc.vector.tensor_copy(
                    qkT_sb[r][:, qc * 512 : (qc + 1) * 512], box["ps"]
                )

            return [lambda i0=i0: mmstep(i0) for i0 in range(0, ND, 2)] + [cpstep]

        def v_group_steps(t, force_mm=False):
            """2 matmul steps (4 accum chunks each) + 1 copy step."""
            nonlocal group_idx
            sel = 0 if force_mm else group_idx % 3
            group_idx += 1
            pool, tag = pools3[sel], tags3[sel]
            box = {}

            def mmstep(i0):
                if i0 == 0:
                    box["ps"] = pool.tile([128, HPG * HD], f32, tag=tag, name=f"ps_v_{t}")
                for i in range(i0, i0 + 4):
                    nc.tensor.matmul(
                        box["ps"],
                        lhsT=xT_sb[i][:, t * 128 : (t + 1) * 128],
                        rhs=wv_sb[i],
                        start=(i == 0),
                        stop=(i == ND - 1),
                    )

            def cpstep():
                vview = v_sb[t].rearrange("p (h c) -> p h c", c=HD + 1)[:, :, 0:HD]
                nc.vector.tensor_copy(vview, box["ps"].rearrange("p (h c) -> p h c", c=HD))

            return [lambda i0=i0: mmstep(i0) for i0 in range(0, ND, 4)] + [cpstep]

        def outproj_piece(c2, t, dc, box):
            """One micro-task: single matmul + copy; the dc=1 task DMAs the
            full [128, 1024] row-block in ONE transfer (the two dc halves
            share an SBUF staging tile), halving y trigger count to 32.
            Triggers alternate sync/gpsimd rings (the scale chain no longer
            uses DMA, so both rings are safe).  Drain-wave pieces CAST via
            the Act engine (idle once the exps are done) so the tail is not
            serialized on DVE."""
            ydst = y0 if c2 == 0 else y1
            trig = nc.sync if t % 2 == 0 else nc.gpsimd

            def step():
                ps = ps_mm.tile([128, 512], f32, tag="mm", name=f"ps_y{c2}_{t}_{dc}")
                nc.tensor.matmul(
                    ps,
                    lhsT=oT_sb[c2][:, t * 128 : (t + 1) * 128],
                    rhs=wo_sb[c2][:, dc * 512 : (dc + 1) * 512],
                    start=True,
                    stop=True,
                )
                if dc == 0:
                    box["ysb"] = ysb_pool.tile(
                        [128, 1024], bf16, tag="y", name=f"ysb{c2}_{t}"
                    )
                ysb = box["ysb"]
                if box.get("act_cast"):
                    nc.scalar.copy(ysb[:, dc * 512 : (dc + 1) * 512], ps)
                else:
                    nc.vector.tensor_copy(ysb[:, dc * 512 : (dc + 1) * 512], ps)
                if dc == 1:
                    trig.dma_start(
                        out=ydst[t * 128 : (t + 1) * 128, :],
                        in_=ysb,
                    )

            return [step]

        # ---- prefix: only what slot 0's scores need ----
        # qT qc0 + kT kb0-3, gated on xT quarter 0 + wqk.  The two groups
        # are interleaved PER CHUNK so the last-arriving input chunk gates
        # only the final MM pair, not a whole second 8-chunk pass.
        # Everything else (including v(0..3)) is spliced into the slots.
        def run_all(steps):
            for s in steps:
                s()

        psQ = ps_mm.tile([128, 512], f32, tag="mm", name="ps_qk_pre0")
        psK = ps_o.tile([128, 512], f32, tag="o", name="ps_qk_pre2")
        for i in range(ND):
            nc.tensor.matmul(
                psQ,
                lhsT=wqk_sb[i][:, 0:128],
                rhs=xT_sb[i][:, 0:512],
                start=(i == 0),
                stop=(i == ND - 1),
            )
            nc.tensor.matmul(
                psK,
                lhsT=wqk_sb[i][:, 2 * 128 : 3 * 128],
                rhs=xT_sb[i][:, 0:512],
                start=(i == 0),
                stop=(i == ND - 1),
            )
        nc.vector.tensor_copy(qkT_sb[0][:, 0:512], psQ)
        nc.vector.tensor_copy(qkT_sb[2][:, 0:512], psK)

        # ---- micro-task queue for the kb-slot loop ----
        # Tasks become eligible at a given global slot; each slot pops ALL
        # eligible tasks in eligibility order (min-heap -- a far-future
        # task must not block an overdue one behind it).
        import heapq

        taskq = []  # heap of (eligible_slot, seq, fn)
        seq_ctr = [0]

        def enq(slot, steps):
            for k, s in enumerate(steps):
                heapq.heappush(taskq, (slot + k, seq_ctr[0], s))
                seq_ctr[0] += 1

        # v blocks during early pair-0 slots.  av_pair(t) is emitted in slot
        # t+1 and pops happen before it, so v(t)'s 3 steps must be popped
        # by slot t (eligibility t-2 with pop-all-eligible).  kT qc2/qc3
        # are READ by the scores emitted at slots 8/12, which precede that
        # slot's pop -- their 5 tasks must pop by slot 4qc-1.  The early
        # slots are PE-production-bound regardless; multi-pop is correct.
        for t in range(NT):
            enq(t - 2, v_group_steps(t, force_mm=True))
        enq(-1, qk_group_steps(2, 1, force_mm=True))      # kT pair0 kb 4-7 (by slot 3)
        enq(3, qk_group_steps(2, 2, force_mm=True))       # kT pair0 kb 8-11
        enq(7, qk_group_steps(2, 3, force_mm=True))       # kT pair0 kb 12-15
        enq(11, qk_group_steps(0, 1, force_mm=True))      # qT pair0 qc1 (slot 16)
        # Pair-1 projections and qT pair0 qc2/3: confined to slots 16-44 so
        # they never share the ps_mm ring with the output-projection pieces
        # (waves start at slot 45) -- a spliced group holds its PSUM buf for
        # ~5 slots and stalls the 2-buf ring if pieces interleave.
        enq(16, qk_group_steps(0, 2, force_mm=True))      # qT pair0 qc2 (slot 32)
        enq(19, qk_group_steps(2 + 1, 0, force_mm=True))  # kT pair1
        enq(22, qk_group_steps(2 + 1, 1, force_mm=True))
        enq(25, qk_group_steps(2 + 1, 2, force_mm=True))
        enq(28, qk_group_steps(2 + 1, 3, force_mm=True))
        enq(31, qk_group_steps(0, 3, force_mm=True))      # qT pair0 qc3 (slot 48)
        enq(34, qk_group_steps(1, 0, force_mm=True))      # qT pair1
        enq(37, qk_group_steps(1, 1, force_mm=True))
        enq(40, qk_group_steps(1, 2, force_mm=True))
        enq(43, qk_group_steps(1, 3, force_mm=True))

        def pop_task(g):
            popped = False
            while taskq and taskq[0][0] <= g:
                _, _, fn = heapq.heappop(taskq)
                fn()
                popped = True
            return popped

        # ---- phase 2: attention, pair-outer ----
        def av_pair(pair, poA, poB, kb, pt):
            hA, hB = 2 * pair, 2 * pair + 1
            nc.tensor.matmul(
                poA,
                lhsT=v_sb[kb][:, hA * (HD + 1) : (hA + 1) * (HD + 1)],
                rhs=pt[:, 0:512],
                start=(kb == 0),
                stop=(kb == NT - 1),
            )
            nc.tensor.matmul(
                poB,
                lhsT=v_sb[kb][:, hB * (HD + 1) : (hB + 1) * (HD + 1)],
                rhs=pt[:, 512:1024],
                start=(kb == 0),
                stop=(kb == NT - 1),
            )

        for pair in range(2):
            for qc in range(NQ):
                seg = pair * NQ + qc
                poA = ps_o.tile([65, 512], f32, tag="o", name=f"poA_{pair}_{qc}")
                poB = ps_o.tile([65, 512], f32, tag="o", name=f"poB_{pair}_{qc}")
                pending = None
                for kb in range(NT):
                    g = seg * NT + kb
                    ps = ps_s.tile(
                        [128, 1024], f32, tag="s", name=f"ps_s_{pair}_{qc}_{kb}"
                    )
                    # Row-split concurrent pair: head A on PE rows 0-63,
                    # head B on rows 64-127.
                    nc.tensor.matmul(
                        ps[:, 0:512],
                        lhsT=qkT_sb[2 + pair][0:64, kb * 128 : (kb + 1) * 128],
                        rhs=qkT_sb[pair][0:64, qc * 512 : (qc + 1) * 512],
                        start=True,
                        stop=True,
                    )
                    nc.tensor.matmul(
                        ps[:, 512:1024],
                        lhsT=qkT_sb[2 + pair][64:128, kb * 128 : (kb + 1) * 128],
                        rhs=qkT_sb[pair][64:128, qc * 512 : (qc + 1) * 512],
                        start=True,
                        stop=True,
                    )
                    pt = pt_pool.tile(
                        [128, 1024], bf16, tag="pt", name=f"pt_{pair}_{qc}_{kb}"
                    )
                    nc.scalar.activation(pt, ps, EXP, scale=HD**-0.5)
                    pop_task(g)
                    if pending is not None:
                        av_pair(pair, poA, poB, *pending)
                    pending = (kb, pt)
                av_pair(pair, poA, poB, *pending)

                # Softmax scale chain, fully ON-CHIP (no DMA, which queued
                # ~5us/hop behind in-flight y pieces; no custom ISA, which
                # this walrus build cannot codegen):
                #   1. den row [1,512] -> [128,4] via 4 tiny PE transposes
                #      (the PE is right here at the segment end anyway)
                #   2. [128,4] reciprocal on DVE (~0.2us, vs 3.3us for the
                #      single-lane row reciprocal)
                #   3. transpose back to a [1,512] row via 4 PE transposes
                #   4. 64-partition broadcast = rank-1 matmul ones64.T @ row
                #   5. scale-mul on gpsimd
                for h, po in ((0, poA), (1, poB)):
                    qb = h * 64
                    oacc = small.tile(
                        [65, 512], f32, tag="oacc", name=f"oacc_{seg}_{h}", bufs=4
                    )
                    nc.vector.tensor_copy(oacc, po)
                    pden = ps_mm.tile([128, 4], f32, tag="mm", name=f"pden_{seg}_{h}")
                    for j in range(4):
                        nc.tensor.transpose(
                            pden[:, j : j + 1],
                            oacc[64:65, 128 * j : 128 * (j + 1)],
                            onescol_f[64:65, 0:1],
                            tile_position=(64, 0),
                        )
                    rout = small.tile([128, 4], bf16, tag="rout", name=f"rout_{seg}_{h}")
                    with nc.allow_low_precision("softmax denom recip, bf16 rep"):
                        nc.vector.reciprocal(rout, pden)
                    rrow_ps = ps_mm.tile([1, 512], bf16, tag="mm", name=f"rrowp_{seg}_{h}")
                    for j in range(4):
                        nc.tensor.transpose(
                            rrow_ps[0:1, 128 * j : 128 * (j + 1)],
                            rout[:, j : j + 1],
                            ident128,
                        )
                    rrow = small.tile([1, 512], bf16, tag="rrow", name=f"rrow_{seg}_{h}")
                    nc.vector.tensor_copy(rrow, rrow_ps)
                    rep_ps = ps_mm.tile([64, 512], f32, tag="mm", name=f"repp_{seg}_{h}")
                    nc.tensor.matmul(
                        rep_ps, lhsT=onescol[0:1, :], rhs=rrow, start=True, stop=True
                    )
                    rep = small.tile([64, 512], bf16, tag="rep", name=f"rep_{seg}_{h}")
                    nc.vector.tensor_copy(rep, rep_ps)
                    # h0's mul on gpsimd, h1's on DVE: the two run
                    # concurrently instead of serializing on gpsimd.
                    mul_eng = nc.gpsimd if h == 0 else nc.vector
                    mul_eng.tensor_mul(
                        oT_sb[pair][qb : qb + 64, qc * 512 : (qc + 1) * 512],
                        oacc[0:64, :],
                        rep,
                    )
                # Output projection pieces: one task/slot in waves that start
                # after the qk splices end (slot 45) AND after this segment's
                # scale chain has landed (~8 slots past segment end --
                # chain latency is ~4-5 slots plus margin; pieces popped
                # earlier stall the in-order PE queue on the gpsimd mul).
                wave = max(45 + 8 * seg, seg * NT + NT + 8)
                for tb in range(4):
                    box = {"act_cast": wave + 2 * tb >= 126}
                    for dc in range(2):
                        enq(
                            wave + 2 * tb + dc,
                            outproj_piece(pair, qc * 4 + tb, dc, box),
                        )

        # Drain any tasks that remained (the last qc's output projection).
        while taskq:
            _, _, fn = heapq.heappop(taskq)
            fn()


def build():
    nc = bass.Bass("TRN2", target_bir_lowering=False)
    xT = nc.dram_tensor("xT", [D, N], bf16, kind="ExternalInput").ap()
    wqk = nc.dram_tensor("wqk", [D, 2 * HPG * HD], bf16, kind="ExternalInput").ap()
    wv = nc.dram_tensor("wv", [D, HPG * HD], bf16, kind="ExternalInput").ap()
    wo = nc.dram_tensor("wo", [2 * 128, D], bf16, kind="ExternalInput").ap()
    y0 = nc.dram_tensor("y0", [N, D], bf16, kind="ExternalOutput").ap()
    y1 = nc.dram_tensor("y1", [N, D], bf16, kind="ExternalOutput").ap()
    with _TC(nc) as tc:
        _body(nc, tc, xT, wqk, wv, wo, y0, y1)
    return nc


def shard_inputs(x, w_qkv, w_out):
    """Build the 8 per-core input maps from the full tensors (bf16)."""
    import ml_dtypes

    bfd = ml_dtypes.bfloat16
    x = np.asarray(x, dtype=np.float32)
    w_qkv = np.asarray(w_qkv, dtype=np.float32)
    w_out = np.asarray(w_out, dtype=np.float32)
    in_maps = []
    for c in range(NCORES):
        b, grp = c // 4, c % 4
        heads = [HPG * grp + i for i in range(HPG)]
        xTa = np.ascontiguousarray(x[b].T).astype(bfd)
        qcols = [w_qkv[:, h * HD : (h + 1) * HD] for h in heads]
        kcols = [w_qkv[:, H * HD + h * HD : H * HD + (h + 1) * HD] for h in heads]
        vcols = [w_qkv[:, 2 * H * HD + h * HD : 2 * H * HD + (h + 1) * HD] for h in heads]
        wqk_a = np.ascontiguousarray(np.concatenate(qcols + kcols, axis=1)).astype(bfd)
        wv_a = np.ascontiguousarray(np.concatenate(vcols, axis=1)).astype(bfd)
        wo_a = np.ascontiguousarray(
            np.concatenate([w_out[h * HD : (h + 1) * HD, :] for h in heads], axis=0)
        ).astype(bfd)
        in_maps.append({"xT": xTa, "wqk": wqk_a, "wv": wv_a, "wo": wo_a})
    return in_maps


LAST_RESULTS = None  # BassKernelResults from the most recent kernel() call
_NC_CACHE = None


def kernel(x, w_qkv, w_out):
    global LAST_RESULTS, _NC_CACHE
    if _NC_CACHE is None:
        _NC_CACHE = build()
    nc = _NC_CACHE
    in_maps = shard_inputs(x, w_qkv, w_out)
    trace = bool(os.environ.get("KERNEL_TRACE"))
    res = bass_utils.run_bass_kernel_spmd(
        nc, in_maps, core_ids=list(range(NCORES)), trace=trace
    )
    LAST_RESULTS = res
    y = np.zeros((B, N, D), dtype=np.float32)
    for c in range(NCORES):
        y[c // 4] += np.asarray(res.results[c]["y0"], dtype=np.float32)
        y[c // 4] += np.asarray(res.results[c]["y1"], dtype=np.float32)
    return y

